# revision 45
# baseline (speedup 1.0000x reference)
"""Bass/Trainium2 kernel for nn_KernelEdges (gnn_message_passing).

Computes A = exp((g_i + g_j - 2*Xf@Xf.T)/sigma^2) with zeroed diagonal,
broadcast to all B batch slots, where Xf = X.transpose(1,0,2).reshape(N, B*d).

A is SYMMETRIC, and this kernel exploits that with a wrapped-window
partition that stays SPMD-uniform: row-block bk (128 rows) computes and
stores only columns [128*bk, 128*bk + 9*128) mod N.  For any block pair
at ring-distance d = (cb - rb) mod 16, either d <= 8 (covered directly)
or 16 - d <= 8 (covered by the transposed partner), so the host mirrors
the missing half from A.T.  Each core handles two adjacent row-blocks,
and because each core's xt copy is column-ROLLED so its own rows sit at
rolled columns 0..255, every core's compute window is the SAME rolled
range [0, 1280) and its store windows the same [0,1152)/[128,1280):
shapes are identical across cores, as SPMD requires.

This cuts, per core: input 2.1->1.31 MB, stores 1.05->0.59 MB (the
single DMA ring that serializes all traffic drops from ~18 to ~11 us of
chain time), and PE column-streaming by 37%.

Per-core device work, per psum chain (mt, g) over column group g of
widths [512, 512, 256] within the rolled window:
  psum = (-1/2*ones).T @ g_row[g]                        (rank-1: -g_j/2)
       + sum_q xt_q[:, mt_cols].T @ xt_q[:, g_cols]      (Gram matrix)
  A    = exp(-2/sigma^2 * psum + g_i/sigma^2)            (ACT, bias/row)
  DMA the per-(mt,g) slice of the store window to DRAM.

Schedule carries over the measured lessons: ONE sync ring, sequential
pieces (h-major so the g0 chains stop early), stores chained behind the
input, rank-1 seeds interleaved, no extra engines/rings (the chip power
throttle punishes any added concurrency).

Diagonal zeroing, mirroring, and the batch broadcast happen on host.
"""

import numpy as np

B, N, D = 8, 2048, 64
NCORES = 8
R = N // NCORES          # 256 rows per core
KD = B * D               # 512 contraction dim
NB = 512                 # psum bank width (fp32)
NMT = R // 128           # 2 m-tiles (row-blocks) per core
NQ = KD // 128           # 4 k-tiles
NBLK = N // 128          # 16 row/col blocks globally

W = 1280                 # compute window (rolled cols), 10 blocks
SW = 1152                # store window per row-block, 9 blocks
# column groups within the window: (offset, width); 512-aligned for psum
GROUPS = [(0, 512), (512, 512), (1024, 256)]
# input piece column-halves (h-major streaming)
HALVES = [(0, 512), (512, 768)]

MM_MODE = "bf16"
OUT_BF16 = True

# store pieces (g, mt): intersection of group g with mt's store window
# [128*mt, 128*mt + SW); (col_off_in_window, width, out_col_off)
def _store_pieces():
    pieces = []
    coff = 0
    for g, (off, w) in enumerate(GROUPS):
        for mt in range(NMT):
            lo = max(off, 128 * mt)
            hi = min(off + w, 128 * mt + SW)
            pieces.append((g, mt, lo, hi - lo, coff))
            coff += hi - lo
    return pieces, coff

STORE_PIECES, OUTW = _store_pieces()   # OUTW = 2304


def _build_program(inv_s2):
    import concourse.bass as bass
    import concourse.tile as tile
    from concourse import bacc, mybir

    f32 = mybir.dt.float32
    mm_dt = mybir.dt.bfloat16 if MM_MODE == "bf16" else mybir.dt.float32r
    out_dt = mybir.dt.bfloat16 if OUT_BF16 else f32

    nc = bacc.Bacc(
        "TRN2", target_bir_lowering=False, debug=False, num_devices=NCORES
    )
    GK = 2 if MM_MODE == "bf16" else 1

    xt_d = nc.dram_tensor("xt", [KD, W], mm_dt, kind="ExternalInput").ap()
    bias_d = nc.dram_tensor("bias", [128, NMT], f32, kind="ExternalInput").ap()
    grow_d = nc.dram_tensor("grow", [GK, W], mm_dt, kind="ExternalInput").ap()
    # out: partition-major [128, OUTW]; piece (g, mt) at its col offset
    out_d = nc.dram_tensor(
        "out", [128, OUTW], out_dt, kind="ExternalOutput"
    ).ap()

    with tile.TileContext(nc) as tc:
        with (
            tc.tile_pool(name="persist", bufs=1) as persist,
            tc.tile_pool(name="apool", bufs=1) as apool,
            tc.tile_pool(name="psum", bufs=1, space="PSUM") as pspool,
        ):
            neg_half = persist.tile([GK, 128], mm_dt, name="neg_half")
            if MM_MODE == "bf16":
                nc.gpsimd.memset(
                    neg_half[:].bitcast(mybir.dt.uint16), 0xBF00
                )
            else:
                nc.gpsimd.memset(
                    neg_half[:].bitcast(mybir.dt.uint32), 0xBF000000
                )
            grow_sb = persist.tile([GK, W], mm_dt, name="grow")
            nc.gpsimd.dma_start(grow_sb[:], grow_d[:])
            bias_sb = persist.tile([128, NMT], f32, name="bias")
            nc.scalar.dma_start(bias_sb[:], bias_d[:])

            # xt pieces (h, q) sequential on the sync ring, h-major
            xt_sb = [
                persist.tile([128, W], mm_dt, name=f"xt{q}")
                for q in range(NQ)
            ]
            # the final (h1, q3) piece is split at the g1/g2 boundary so
            # the g1 chains stop as soon as their own columns land instead
            # of waiting for g2's tail of the merged transfer
            for hoff, hw_ in HALVES:
                for q in range(NQ):
                    segs = (
                        [(hoff, 512), (hoff + 512, hw_ - 512)]
                        if (hoff, q) == (512, NQ - 1)
                        else [(hoff, hw_)]
                    )
                    for so, sw in segs:
                        nc.sync.dma_start(
                            xt_sb[q][:, so:so + sw],
                            xt_d[q * 128:(q + 1) * 128, so:so + sw],
                        )

            # each chain is CLIPPED to its row-block's store window
            # [128*mt, 128*mt+SW): mt0 never needs g2's cols [1152:1280)
            # and mt1 never needs g0's cols [0:128) - dropping them saves
            # 1280 PE column-units and narrows two ACTs for free
            clip = {}
            for g, (off, w) in enumerate(GROUPS):
                for mt in range(NMT):
                    lo = max(off, 128 * mt)
                    hi = min(off + w, 128 * mt + SW)
                    clip[mt, g] = (lo, hi - lo)
            ps = {
                (mt, g): pspool.tile(
                    [128, clip[mt, g][1]], f32, name=f"ps{mt}{g}"
                )
                for g in range(len(GROUPS)) for mt in range(NMT)
            }
            a_sb = {
                mt: apool.tile([128, W], out_dt, name=f"a{mt}")
                for mt in range(NMT)
            }
            # matmuls h-major: group g0 lives in half 0, g1/g2 in half 1.
            # Seeds (rank-1 g_j) interleave per-half; LHS is the core's
            # own 256 rolled columns (inside half 0, always loaded first)
            half_groups = [[0], [1, 2]]
            for h, glist in enumerate(half_groups):
                for g in glist:
                    for mt in range(NMT):
                        lo, w = clip[mt, g]
                        nc.tensor.matmul(
                            ps[mt, g][:],
                            neg_half[:],
                            grow_sb[:, lo:lo + w],
                            start=True,
                            stop=False,
                        )
                for q in range(NQ):
                    for mt in range(NMT):
                        for g in glist:
                            lo, w = clip[mt, g]
                            nc.tensor.matmul(
                                ps[mt, g][:],
                                xt_sb[q][:, mt * 128:(mt + 1) * 128],
                                xt_sb[q][:, lo:lo + w],
                                start=False,
                                stop=(q == NQ - 1),
                            )
            # ACT + store per (g, mt) in stop order; stores chain on sync
            for g, mt, lo, w, coff in STORE_PIECES:
                # clip == store piece exactly, so ACT width == store width
                nc.scalar.activation(
                    a_sb[mt][:, lo:lo + w],
                    ps[mt, g][:],
                    mybir.ActivationFunctionType.Exp,
                    bias=bias_sb[:, mt:mt + 1],
                    scale=-2.0 * inv_s2,
                )
                nc.sync.dma_start(
                    out_d[:, coff:coff + w], a_sb[mt][:, lo:lo + w]
                )

    nc.compile()
    return nc


def _prepare(X, log_sigma):
    import ml_dtypes

    X = np.ascontiguousarray(X, dtype=np.float32)
    assert X.shape == (B, N, D), X.shape
    sigma = float(np.exp(np.float32(log_sigma)))
    inv_s2 = 1.0 / (sigma * sigma)

    XT = np.ascontiguousarray(X.transpose(0, 2, 1).reshape(KD, N))
    g = np.einsum("kn,kn->n", XT, XT).astype(np.float32)

    mm_np = ml_dtypes.bfloat16 if MM_MODE == "bf16" else np.float32
    XTm = XT.astype(mm_np)

    in_maps = []
    for c in range(NCORES):
        r0 = c * R
        Xr = np.roll(XTm, -r0, axis=1)[:, :W]
        gr = np.roll(g, -r0)[:W]
        bias_np = np.empty((128, NMT), dtype=np.float32)
        for mt in range(NMT):
            bias_np[:, mt] = g[r0 + mt * 128: r0 + (mt + 1) * 128] * inv_s2
        if MM_MODE == "bf16":
            g_hi = gr.astype(ml_dtypes.bfloat16)
            g_lo = (gr - g_hi.astype(np.float32)).astype(ml_dtypes.bfloat16)
            grow_np = np.ascontiguousarray(np.stack([g_hi, g_lo]))
        else:
            grow_np = np.ascontiguousarray(gr[None, :])
        in_maps.append({
            "xt": np.ascontiguousarray(Xr),
            "bias": bias_np,
            "grow": grow_np,
        })
    return inv_s2, in_maps


def kernel(X, log_sigma):
    from concourse.bass_utils import run_bass_kernel_spmd

    inv_s2, in_maps = _prepare(X, log_sigma)
    nc = _build_program(inv_s2)
    res = run_bass_kernel_spmd(nc, in_maps, list(range(NCORES)))

    A = np.empty((N, N), dtype=np.float32)
    for c in range(NCORES):
        r0 = c * R
        o = np.asarray(res.results[c]["out"]).astype(np.float32)  # [128,OUTW]
        for mt in range(NMT):
            # reassemble this row-block's [128, SW] store window from its
            # per-group pieces (window = rolled cols [128*mt, 128*mt+SW))
            t = np.empty((128, SW), dtype=np.float32)
            for g, pmt, lo, w, coff in STORE_PIECES:
                if pmt != mt:
                    continue
                t[:, lo - 128 * mt: lo - 128 * mt + w] = o[:, coff:coff + w]
            rows = slice(r0 + mt * 128, r0 + (mt + 1) * 128)
            cols = (np.arange(SW) + 128 * mt + r0) % N
            A[rows, cols] = t
    # mirror the uncovered half: block pair at ring-distance d>8 comes
    # from its transposed partner (which has distance 16-d <= 8)
    for a in range(NBLK):
        for b in range(NBLK):
            if (b - a) % NBLK > 8:
                A[a * 128:(a + 1) * 128, b * 128:(b + 1) * 128] = \
                    A[b * 128:(b + 1) * 128, a * 128:(a + 1) * 128].T
    idx = np.arange(N)
    A[idx, idx] = 0.0
    out = np.empty((B, N, N), dtype=np.float32)
    out[:] = A[None, :, :]
    return out
